# revision 10
# baseline (speedup 1.0000x reference)
"""Trainium2 Bass kernel: 8-expert top-2 MoE layer (SwiGLU experts).

Sharding: expert parallelism across 8 NeuronCores. The host performs the
all-to-all token dispatch as part of input sharding (gather the tokens routed
to each expert, ship them transposed to that expert's core) and the combine
scatter-add as part of output unsharding. All model math runs on device:
router logits (exact fp32 matmuls), softmax top-2 renormalized combine
weights, the expert FFN (float32r matmuls at full PE rate), and the
combine-weight scaling.

Self-contained: hardcodes all shapes from the problem spec.
"""

import os

import numpy as np

# Problem constants
H = 1024  # hidden dim
I = 4096  # intermediate dim
E = 8  # experts
P = 128  # SBUF partitions

# Tiling constants
TB = 512  # tokens per block (matmul moving free dim)
TS = TB // P  # token subtiles per block
IS = 1024  # intermediate features resident per weight chunk
N_SUPER = I // IS
IT = IS // P  # i-tiles per super chunk
HO = H // P  # h chunks (contraction tiles)
HH = H // 512  # output column halves for the down projection


def build_moe(Tc: int, mm_dt_name: str = "float32r"):
    """Build the per-core Bass program for Tc tokens (Tc % 512 == 0)."""
    import concourse.bass as bass  # noqa: F401
    import concourse.mybir as mybir
    import concourse.tile as tile
    from concourse import bacc

    assert Tc % TB == 0
    NB = Tc // TB
    f32 = mybir.dt.float32
    mm_dt = getattr(mybir.dt, mm_dt_name)
    Alu = mybir.AluOpType
    Act = mybir.ActivationFunctionType
    X = mybir.AxisListType.X

    nc = bacc.Bacc(
        "TRN2", target_bir_lowering=False, debug=False, num_devices=8
    )

    xT = nc.dram_tensor("xT", [H, Tc], mm_dt, kind="ExternalInput").ap()
    xTr = nc.dram_tensor("xTr", [H, Tc], f32, kind="ExternalInput").ap()
    wg = nc.dram_tensor("wg", [H, I], mm_dt, kind="ExternalInput").ap()
    wu = nc.dram_tensor("wu", [H, I], mm_dt, kind="ExternalInput").ap()
    wd = nc.dram_tensor("wd", [I, H], mm_dt, kind="ExternalInput").ap()
    gpT = nc.dram_tensor("gpT", [H, E], f32, kind="ExternalInput").ap()
    sel = nc.dram_tensor("sel", [P, E], f32, kind="ExternalInput").ap()
    out = nc.dram_tensor("out", [Tc, H], f32, kind="ExternalOutput").ap()

    # Partition-major views: h (or i) split as outer*P + partition
    xT_r = xT.rearrange("(ho p) t -> p ho t", p=P)  # [128, 8, Tc]
    xTr_r = xTr.rearrange("(ho p) t -> p ho t", p=P)  # [128, 8, Tc]
    wg_r = wg.rearrange("(ho p) i -> p ho i", p=P)  # [128, 8, 4096]
    wu_r = wu.rearrange("(ho p) i -> p ho i", p=P)
    wd_r = wd.rearrange("(io p) h -> p io h", p=P)  # [128, 32, 1024]
    gpT_r = gpT.rearrange("(ho p) e -> p ho e", p=P)  # [128, 8, 8]



    with tile.TileContext(nc) as tc:
        with (
            tc.tile_pool(name="singles", bufs=1) as singles,
            tc.tile_pool(name="weights", bufs=1) as wpool,
            tc.tile_pool(name="xp", bufs=2) as xpool,
            tc.tile_pool(name="hp", bufs=2) as hpool,
            tc.tile_pool(name="sp", bufs=4) as spool,
            tc.tile_pool(name="op", bufs=3) as opool,
            tc.tile_pool(name="pgu", bufs=2, space="PSUM") as pgu,
            tc.tile_pool(name="pout", bufs=2, space="PSUM") as pout,
            tc.tile_pool(name="pr", bufs=2, space="PSUM") as pr,
        ):
            gp_sb = singles.tile([P, HO, E], f32)
            nc.sync.dma_start(gp_sb, gpT_r)
            sel_sb = singles.tile([P, E], f32)
            nc.sync.dma_start(sel_sb, sel)
            # Per-token renormalized top-2 combine weight for this expert;
            # written once (first weight chunk), read by every chunk.
            w_all = singles.tile([P, NB * TS], f32)

            for sup in range(N_SUPER):
                i0 = sup * IS
                wg_sb = wpool.tile([P, HO, IS], mm_dt, tag="wg")
                nc.sync.dma_start(wg_sb, wg_r[:, :, i0 : i0 + IS])
                wu_sb = wpool.tile([P, HO, IS], mm_dt, tag="wu")
                nc.sync.dma_start(wu_sb, wu_r[:, :, i0 : i0 + IS])
                wd_sb = wpool.tile([P, IT, H], mm_dt, tag="wd")
                nc.sync.dma_start(wd_sb, wd_r[:, sup * IT : (sup + 1) * IT, :])

                for blk in range(NB):
                    t0 = blk * TB
                    x_sb = xpool.tile([P, HO, TB], mm_dt, tag="x")
                    nc.sync.dma_start(x_sb, xT_r[:, :, t0 : t0 + TB])

                    if sup == 0:
                        # Router: logits -> renormalized top-2 weight for
                        # this core's expert, all in fp32 for exact top-2
                        # selection vs the reference.
                        for tsub in range(TS):
                            col = blk * TS + tsub
                            c0 = t0 + tsub * P
                            xr = xpool.tile([P, HO, P], f32, tag="xr")
                            nc.sync.dma_start(xr, xTr_r[:, :, c0 : c0 + P])
                            lg = pr.tile([P, E], f32, tag="lg")
                            for ho in range(HO):
                                nc.tensor.matmul(
                                    lg,
                                    lhsT=xr[:, ho, :],
                                    rhs=gp_sb[:, ho, :],
                                    start=(ho == 0),
                                    stop=(ho == HO - 1),
                                )
                            m_sb = spool.tile([P, 1], f32, tag="m")
                            nc.vector.reduce_max(out=m_sb, in_=lg, axis=X)
                            negm = spool.tile([P, 1], f32, tag="negm")
                            nc.vector.reduce_max(
                                out=negm, in_=lg, axis=X, negate=True
                            )
                            q = spool.tile([P, E], f32, tag="q")
                            nc.scalar.activation(q, lg, Act.Exp, bias=negm)
                            # mask of non-top1 entries (l < max)
                            nm = spool.tile([P, E], f32, tag="nm")
                            nc.vector.tensor_scalar(
                                nm, lg, m_sb, None, op0=Alu.is_lt
                            )
                            qm = spool.tile([P, E], f32, tag="qm")
                            nc.vector.tensor_tensor(qm, q, nm, op=Alu.mult)
                            q2 = spool.tile([P, 1], f32, tag="q2")
                            nc.vector.reduce_max(out=q2, in_=qm, axis=X)
                            qs = spool.tile([P, E], f32, tag="qs")
                            nc.vector.tensor_tensor(qs, q, sel_sb, op=Alu.mult)
                            qe = spool.tile([P, 1], f32, tag="qe")
                            nc.vector.reduce_sum(out=qe, in_=qs, axis=X)
                            ind = spool.tile([P, 1], f32, tag="ind")
                            nc.vector.tensor_tensor(ind, qe, q2, op=Alu.is_ge)
                            den = spool.tile([P, 1], f32, tag="den")
                            nc.vector.tensor_scalar_add(den, q2, 1.0)
                            rec = spool.tile([P, 1], f32, tag="rec")
                            nc.vector.reciprocal(rec, den)
                            num = spool.tile([P, 1], f32, tag="num")
                            nc.vector.tensor_tensor(num, qe, ind, op=Alu.mult)
                            nc.vector.tensor_tensor(
                                w_all[:, col : col + 1], num, rec, op=Alu.mult
                            )

                    # Expert FFN for this (i-chunk, token block):
                    # hT[i, t] = silu(Wg.T x)[i, t] * (Wu.T x)[i, t]
                    h_sb = hpool.tile([P, IT, TB], mm_dt, tag="h")
                    for it in range(IT):
                        gps = pgu.tile([P, TB], f32, tag="g")
                        ups = pgu.tile([P, TB], f32, tag="u")
                        for ho in range(HO):
                            nc.tensor.matmul(
                                gps,
                                lhsT=wg_sb[:, ho, it * P : (it + 1) * P],
                                rhs=x_sb[:, ho, :],
                                start=(ho == 0),
                                stop=(ho == HO - 1),
                            )
                        for ho in range(HO):
                            nc.tensor.matmul(
                                ups,
                                lhsT=wu_sb[:, ho, it * P : (it + 1) * P],
                                rhs=x_sb[:, ho, :],
                                start=(ho == 0),
                                stop=(ho == HO - 1),
                            )
                        sig = opool.tile([P, TB], f32, tag="sig")
                        nc.scalar.activation(sig, gps, Act.Sigmoid)
                        gs = opool.tile([P, TB], f32, tag="gs")
                        nc.vector.tensor_tensor(gs, sig, gps, op=Alu.mult)
                        nc.vector.tensor_tensor(
                            h_sb[:, it, :], gs, ups, op=Alu.mult
                        )

                    # Down projection back to token-partition layout, scaled
                    # by the combine weight at PSUM eviction; accumulate
                    # partial sums over i-chunks directly in DRAM.
                    for tsub in range(TS):
                        col = blk * TS + tsub
                        r0 = t0 + tsub * P
                        for hh in range(HH):
                            ops = pout.tile([P, 512], f32, tag="o")
                            for it in range(IT):
                                nc.tensor.matmul(
                                    ops,
                                    lhsT=h_sb[:, it, tsub * P : (tsub + 1) * P],
                                    rhs=wd_sb[:, it, hh * 512 : (hh + 1) * 512],
                                    start=(it == 0),
                                    stop=(it == IT - 1),
                                )
                            oev = opool.tile([P, 512], f32, tag="oev")
                            nc.vector.tensor_scalar_mul(
                                oev, ops, w_all[:, col : col + 1]
                            )
                            nc.gpsimd.dma_start(
                                out[r0 : r0 + P, hh * 512 : (hh + 1) * 512],
                                oev,
                                accum_op=(
                                    Alu.bypass if sup == 0 else Alu.add
                                ),
                            )

    nc.compile()
    return nc


def _run_spmd(nc, in_maps, trace):
    from concourse import bass_utils

    if trace:
        try:
            res = bass_utils.run_bass_kernel_spmd(
                nc, in_maps, core_ids=list(range(E)), trace=True
            )
            if res.exec_time_ns is not None:
                print(f"HW exec time: {res.exec_time_ns} ns")
            return res
        except Exception as exc:  # fall back to an untraced run
            print(f"traced run failed ({exc!r}); retrying without trace")
    return bass_utils.run_bass_kernel_spmd(
        nc, in_maps, core_ids=list(range(E)), trace=False
    )


def prepare(hidden_states, gate_proj_w, gate_weights, up_weights, down_weights,
            mode="sparse", mm_dt_name="float32r"):
    """Shard inputs per strategy; returns (nc, in_maps, combine_fn)."""
    x = np.ascontiguousarray(hidden_states, dtype=np.float32)
    gpw = np.ascontiguousarray(gate_proj_w, dtype=np.float32)
    T = x.shape[0]
    gpT = np.ascontiguousarray(gpw.T)  # [H, E]
    sel_maps = [
        np.tile(np.eye(E, dtype=np.float32)[e], (P, 1)) for e in range(E)
    ]

    def expert_map(e, xTe):
        return {
            "xT": xTe,
            "xTr": xTe,
            "wg": np.ascontiguousarray(gate_weights[e], np.float32),
            "wu": np.ascontiguousarray(up_weights[e], np.float32),
            "wd": np.ascontiguousarray(down_weights[e], np.float32),
            "gpT": gpT,
            "sel": sel_maps[e],
        }

    if mode == "dense":
        xT = np.ascontiguousarray(x.T)  # [H, T]
        nc = build_moe(T, mm_dt_name)
        in_maps = [expert_map(e, xT) for e in range(E)]

        def combine(results):
            outs = [results[e]["out"] for e in range(E)]
            return np.sum(np.stack(outs, axis=0), axis=0).astype(np.float32)

        return nc, in_maps, combine

    # Sparse mode: host-side all-to-all dispatch. Routing decisions here are
    # only used to decide which tokens ship to which expert core; the combine
    # weights themselves are recomputed on device.
    logits = x @ gpw.T  # [T, E] fp32
    top2 = np.argpartition(-logits, 2, axis=1)[:, :2]
    smask = np.zeros((T, E), dtype=bool)
    smask[np.arange(T)[:, None], top2] = True
    idx = [np.nonzero(smask[:, e])[0] for e in range(E)]
    mx = max(len(ix) for ix in idx)
    Tc = max(TB, ((mx + TB - 1) // TB) * TB)

    nc = build_moe(Tc, mm_dt_name)
    in_maps = []
    for e in range(E):
        xTe = np.zeros((H, Tc), dtype=np.float32)
        if len(idx[e]):
            xTe[:, : len(idx[e])] = x[idx[e]].T
        in_maps.append(expert_map(e, xTe))

    def combine(results):
        out = np.zeros((T, H), dtype=np.float32)
        for e in range(E):
            n_e = len(idx[e])
            if n_e:
                out[idx[e]] += results[e]["out"][:n_e]
        return out

    return nc, in_maps, combine


def kernel(hidden_states, gate_proj_w, gate_weights, up_weights, down_weights):
    mode = os.environ.get("MOE_MODE", "sparse")
    mm_dt_name = os.environ.get("MOE_MMDT", "float32r")
    trace = os.environ.get("MOE_TRACE", "0") == "1"
    nc, in_maps, combine = prepare(
        hidden_states, gate_proj_w, gate_weights, up_weights, down_weights,
        mode=mode, mm_dt_name=mm_dt_name,
    )
    res = _run_spmd(nc, in_maps, trace)
    return combine(res.results)


# revision 11
# speedup vs baseline: 1.0051x; 1.0051x over previous
"""Trainium2 Bass kernel: 8-expert top-2 MoE layer (SwiGLU experts).

Sharding: expert parallelism across 8 NeuronCores. The host performs the
all-to-all token dispatch as part of input sharding (gather the tokens routed
to each expert, ship them transposed to that expert's core) and the combine
scatter-add as part of output unsharding. All model math runs on device:
router logits (exact fp32 matmuls), softmax top-2 renormalized combine
weights, the expert FFN (float32r matmuls at full PE rate), and the
combine-weight scaling.

Self-contained: hardcodes all shapes from the problem spec.
"""

import os

import numpy as np

# Problem constants
H = 1024  # hidden dim
I = 4096  # intermediate dim
E = 8  # experts
P = 128  # SBUF partitions

# Tiling constants
TB = 512  # tokens per block (matmul moving free dim)
TS = TB // P  # token subtiles per block
IS = 1024  # intermediate features resident per weight chunk
N_SUPER = I // IS
IT = IS // P  # i-tiles per super chunk
HO = H // P  # h chunks (contraction tiles)
HH = H // 512  # output column halves for the down projection


def build_moe(Tc: int, mm_dt_name: str = "float32r", use_silu: bool = True):
    """Build the per-core Bass program for Tc tokens (Tc % 512 == 0)."""
    import concourse.bass as bass  # noqa: F401
    import concourse.mybir as mybir
    import concourse.tile as tile
    from concourse import bacc

    assert Tc % TB == 0
    NB = Tc // TB
    f32 = mybir.dt.float32
    mm_dt = getattr(mybir.dt, mm_dt_name)
    Alu = mybir.AluOpType
    Act = mybir.ActivationFunctionType
    X = mybir.AxisListType.X

    nc = bacc.Bacc(
        "TRN2", target_bir_lowering=False, debug=False, num_devices=8
    )

    xT = nc.dram_tensor("xT", [H, Tc], mm_dt, kind="ExternalInput").ap()
    xTr = nc.dram_tensor("xTr", [H, Tc], f32, kind="ExternalInput").ap()
    wg = nc.dram_tensor("wg", [H, I], mm_dt, kind="ExternalInput").ap()
    wu = nc.dram_tensor("wu", [H, I], mm_dt, kind="ExternalInput").ap()
    wd = nc.dram_tensor("wd", [I, H], mm_dt, kind="ExternalInput").ap()
    gpT = nc.dram_tensor("gpT", [H, E], f32, kind="ExternalInput").ap()
    sel = nc.dram_tensor("sel", [P, E], f32, kind="ExternalInput").ap()
    out = nc.dram_tensor("out", [Tc, H], f32, kind="ExternalOutput").ap()

    # Partition-major views: h (or i) split as outer*P + partition
    xT_r = xT.rearrange("(ho p) t -> p ho t", p=P)  # [128, 8, Tc]
    xTr_r = xTr.rearrange("(ho p) t -> p ho t", p=P)  # [128, 8, Tc]
    wg_r = wg.rearrange("(ho p) i -> p ho i", p=P)  # [128, 8, 4096]
    wu_r = wu.rearrange("(ho p) i -> p ho i", p=P)
    wd_r = wd.rearrange("(io p) h -> p io h", p=P)  # [128, 32, 1024]
    gpT_r = gpT.rearrange("(ho p) e -> p ho e", p=P)  # [128, 8, 8]



    with tile.TileContext(nc) as tc:
        with (
            tc.tile_pool(name="singles", bufs=1) as singles,
            tc.tile_pool(name="weights", bufs=1) as wpool,
            tc.tile_pool(name="xp", bufs=2) as xpool,
            tc.tile_pool(name="hp", bufs=2) as hpool,
            tc.tile_pool(name="sp", bufs=4) as spool,
            tc.tile_pool(name="op", bufs=3) as opool,
            tc.tile_pool(name="pgu", bufs=2, space="PSUM") as pgu,
            tc.tile_pool(name="pout", bufs=2, space="PSUM") as pout,
            tc.tile_pool(name="pr", bufs=2, space="PSUM") as pr,
        ):
            gp_sb = singles.tile([P, HO, E], f32)
            nc.sync.dma_start(gp_sb, gpT_r)
            sel_sb = singles.tile([P, E], f32)
            nc.sync.dma_start(sel_sb, sel)
            # Per-token renormalized top-2 combine weight for this expert;
            # written once (first weight chunk), read by every chunk.
            w_all = singles.tile([P, NB * TS], f32)

            for sup in range(N_SUPER):
                i0 = sup * IS
                # Per-i-tile weight chunks: compute can start as soon as the
                # first chunk lands, and next-super reloads overlap compute.
                wg_sb, wu_sb, wd_sb = [], [], []
                for it in range(IT):
                    c = i0 + it * P
                    wgc = wpool.tile([P, HO, P], mm_dt, tag=f"wg{it}")
                    nc.sync.dma_start(wgc, wg_r[:, :, c : c + P])
                    wg_sb.append(wgc)
                    wuc = wpool.tile([P, HO, P], mm_dt, tag=f"wu{it}")
                    nc.sync.dma_start(wuc, wu_r[:, :, c : c + P])
                    wu_sb.append(wuc)
                    wdc = wpool.tile([P, H], mm_dt, tag=f"wd{it}")
                    nc.sync.dma_start(wdc, wd_r[:, sup * IT + it, :])
                    wd_sb.append(wdc)

                def down_phase(blk, h_sb, sup=sup, wd_sb=wd_sb):
                    # Down projection back to token-partition layout, scaled
                    # by the combine weight at PSUM eviction; partial sums
                    # over i-chunks accumulate directly in DRAM.
                    t0 = blk * TB
                    for tsub in range(TS):
                        col = blk * TS + tsub
                        r0 = t0 + tsub * P
                        for hh in range(HH):
                            ops = pout.tile([P, 512], f32, tag="o")
                            for it in range(IT):
                                nc.tensor.matmul(
                                    ops,
                                    lhsT=h_sb[:, it, tsub * P : (tsub + 1) * P],
                                    rhs=wd_sb[it][:, hh * 512 : (hh + 1) * 512],
                                    start=(it == 0),
                                    stop=(it == IT - 1),
                                )
                            oev = opool.tile([P, 512], f32, tag="oev")
                            nc.vector.tensor_scalar_mul(
                                oev, ops, w_all[:, col : col + 1]
                            )
                            nc.gpsimd.dma_start(
                                out[r0 : r0 + P, hh * 512 : (hh + 1) * 512],
                                oev,
                                accum_op=(
                                    Alu.bypass if sup == 0 else Alu.add
                                ),
                            )

                pending = None
                for blk in range(NB):
                    t0 = blk * TB
                    x_sb = xpool.tile([P, HO, TB], mm_dt, tag="x")
                    nc.sync.dma_start(x_sb, xT_r[:, :, t0 : t0 + TB])

                    if sup == 0:
                        # Router: logits -> renormalized top-2 weight for
                        # this core's expert, all in fp32 for exact top-2
                        # selection vs the reference.
                        for tsub in range(TS):
                            col = blk * TS + tsub
                            c0 = t0 + tsub * P
                            xr = xpool.tile([P, HO, P], f32, tag="xr")
                            nc.sync.dma_start(xr, xTr_r[:, :, c0 : c0 + P])
                            lg = pr.tile([P, E], f32, tag="lg")
                            for ho in range(HO):
                                nc.tensor.matmul(
                                    lg,
                                    lhsT=xr[:, ho, :],
                                    rhs=gp_sb[:, ho, :],
                                    start=(ho == 0),
                                    stop=(ho == HO - 1),
                                )
                            m_sb = spool.tile([P, 1], f32, tag="m")
                            nc.vector.reduce_max(out=m_sb, in_=lg, axis=X)
                            negm = spool.tile([P, 1], f32, tag="negm")
                            nc.vector.reduce_max(
                                out=negm, in_=lg, axis=X, negate=True
                            )
                            q = spool.tile([P, E], f32, tag="q")
                            nc.scalar.activation(q, lg, Act.Exp, bias=negm)
                            # mask of non-top1 entries (l < max)
                            nm = spool.tile([P, E], f32, tag="nm")
                            nc.vector.tensor_scalar(
                                nm, lg, m_sb, None, op0=Alu.is_lt
                            )
                            qm = spool.tile([P, E], f32, tag="qm")
                            nc.vector.tensor_tensor(qm, q, nm, op=Alu.mult)
                            q2 = spool.tile([P, 1], f32, tag="q2")
                            nc.vector.reduce_max(out=q2, in_=qm, axis=X)
                            qs = spool.tile([P, E], f32, tag="qs")
                            nc.vector.tensor_tensor(qs, q, sel_sb, op=Alu.mult)
                            qe = spool.tile([P, 1], f32, tag="qe")
                            nc.vector.reduce_sum(out=qe, in_=qs, axis=X)
                            ind = spool.tile([P, 1], f32, tag="ind")
                            nc.vector.tensor_tensor(ind, qe, q2, op=Alu.is_ge)
                            den = spool.tile([P, 1], f32, tag="den")
                            nc.vector.tensor_scalar_add(den, q2, 1.0)
                            rec = spool.tile([P, 1], f32, tag="rec")
                            nc.vector.reciprocal(rec, den)
                            num = spool.tile([P, 1], f32, tag="num")
                            nc.vector.tensor_tensor(num, qe, ind, op=Alu.mult)
                            nc.vector.tensor_tensor(
                                w_all[:, col : col + 1], num, rec, op=Alu.mult
                            )

                    # Expert FFN for this (i-chunk, token block):
                    # hT[i, t] = silu(Wg.T x)[i, t] * (Wu.T x)[i, t]
                    h_sb = hpool.tile([P, IT, TB], mm_dt, tag="h")
                    for it in range(IT):
                        gps = pgu.tile([P, TB], f32, tag="g")
                        ups = pgu.tile([P, TB], f32, tag="u")
                        for ho in range(HO):
                            nc.tensor.matmul(
                                gps,
                                lhsT=wg_sb[it][:, ho, :],
                                rhs=x_sb[:, ho, :],
                                start=(ho == 0),
                                stop=(ho == HO - 1),
                            )
                        for ho in range(HO):
                            nc.tensor.matmul(
                                ups,
                                lhsT=wu_sb[it][:, ho, :],
                                rhs=x_sb[:, ho, :],
                                start=(ho == 0),
                                stop=(ho == HO - 1),
                            )
                        if use_silu:
                            gs = opool.tile([P, TB], f32, tag="gs")
                            nc.scalar.activation(gs, gps, Act.Silu)
                        else:
                            sig = opool.tile([P, TB], f32, tag="sig")
                            nc.scalar.activation(sig, gps, Act.Sigmoid)
                            gs = opool.tile([P, TB], f32, tag="gs")
                            nc.vector.tensor_tensor(gs, sig, gps, op=Alu.mult)
                        nc.vector.tensor_tensor(
                            h_sb[:, it, :], gs, ups, op=Alu.mult
                        )

                    # Software pipeline: emit the previous block's down
                    # phase after this block's h production, so the PE's
                    # down matmuls overlap DVE/ACT h work for the next block.
                    if pending is not None:
                        down_phase(*pending)
                    pending = (blk, h_sb)
                if pending is not None:
                    down_phase(*pending)

    nc.compile()
    return nc


def _run_spmd(nc, in_maps, trace):
    from concourse import bass_utils

    if trace:
        try:
            res = bass_utils.run_bass_kernel_spmd(
                nc, in_maps, core_ids=list(range(E)), trace=True
            )
            if res.exec_time_ns is not None:
                print(f"HW exec time: {res.exec_time_ns} ns")
            return res
        except Exception as exc:  # fall back to an untraced run
            print(f"traced run failed ({exc!r}); retrying without trace")
    return bass_utils.run_bass_kernel_spmd(
        nc, in_maps, core_ids=list(range(E)), trace=False
    )


def prepare(hidden_states, gate_proj_w, gate_weights, up_weights, down_weights,
            mode="sparse", mm_dt_name="float32r", use_silu=True):
    """Shard inputs per strategy; returns (nc, in_maps, combine_fn)."""
    x = np.ascontiguousarray(hidden_states, dtype=np.float32)
    gpw = np.ascontiguousarray(gate_proj_w, dtype=np.float32)
    T = x.shape[0]
    gpT = np.ascontiguousarray(gpw.T)  # [H, E]
    sel_maps = [
        np.tile(np.eye(E, dtype=np.float32)[e], (P, 1)) for e in range(E)
    ]

    def expert_map(e, xTe):
        return {
            "xT": xTe,
            "xTr": xTe,
            "wg": np.ascontiguousarray(gate_weights[e], np.float32),
            "wu": np.ascontiguousarray(up_weights[e], np.float32),
            "wd": np.ascontiguousarray(down_weights[e], np.float32),
            "gpT": gpT,
            "sel": sel_maps[e],
        }

    if mode == "dense":
        xT = np.ascontiguousarray(x.T)  # [H, T]
        nc = build_moe(T, mm_dt_name, use_silu)
        in_maps = [expert_map(e, xT) for e in range(E)]

        def combine(results):
            outs = [results[e]["out"] for e in range(E)]
            return np.sum(np.stack(outs, axis=0), axis=0).astype(np.float32)

        return nc, in_maps, combine

    # Sparse mode: host-side all-to-all dispatch. Routing decisions here are
    # only used to decide which tokens ship to which expert core; the combine
    # weights themselves are recomputed on device.
    logits = x @ gpw.T  # [T, E] fp32
    top2 = np.argpartition(-logits, 2, axis=1)[:, :2]
    smask = np.zeros((T, E), dtype=bool)
    smask[np.arange(T)[:, None], top2] = True
    idx = [np.nonzero(smask[:, e])[0] for e in range(E)]
    mx = max(len(ix) for ix in idx)
    Tc = max(TB, ((mx + TB - 1) // TB) * TB)

    nc = build_moe(Tc, mm_dt_name, use_silu)
    in_maps = []
    for e in range(E):
        xTe = np.zeros((H, Tc), dtype=np.float32)
        if len(idx[e]):
            xTe[:, : len(idx[e])] = x[idx[e]].T
        in_maps.append(expert_map(e, xTe))

    def combine(results):
        out = np.zeros((T, H), dtype=np.float32)
        for e in range(E):
            n_e = len(idx[e])
            if n_e:
                out[idx[e]] += results[e]["out"][:n_e]
        return out

    return nc, in_maps, combine


def kernel(hidden_states, gate_proj_w, gate_weights, up_weights, down_weights):
    mode = os.environ.get("MOE_MODE", "sparse")
    mm_dt_name = os.environ.get("MOE_MMDT", "float32r")
    trace = os.environ.get("MOE_TRACE", "0") == "1"
    use_silu = os.environ.get("MOE_SILU", "1") == "1"
    nc, in_maps, combine = prepare(
        hidden_states, gate_proj_w, gate_weights, up_weights, down_weights,
        mode=mode, mm_dt_name=mm_dt_name, use_silu=use_silu,
    )
    res = _run_spmd(nc, in_maps, trace)
    return combine(res.results)


# revision 12
# speedup vs baseline: 1.0767x; 1.0712x over previous
"""Trainium2 Bass kernel: 8-expert top-2 MoE layer (SwiGLU experts).

Sharding: expert parallelism across 8 NeuronCores. The host performs the
all-to-all token dispatch as part of input sharding (gather the tokens routed
to each expert, ship them transposed to that expert's core) and the combine
scatter-add as part of output unsharding. All model math runs on device:
router logits (exact fp32 matmuls), softmax top-2 renormalized combine
weights, the expert FFN (float32r matmuls at full PE rate), and the
combine-weight scaling.

Self-contained: hardcodes all shapes from the problem spec.
"""

import os

import numpy as np

# Problem constants
H = 1024  # hidden dim
I = 4096  # intermediate dim
E = 8  # experts
P = 128  # SBUF partitions

# Tiling constants
TB = 512  # tokens per block (matmul moving free dim)
TS = TB // P  # token subtiles per block
IS = 1024  # intermediate features resident per weight chunk
N_SUPER = I // IS
IT = IS // P  # i-tiles per super chunk
HO = H // P  # h chunks (contraction tiles)
HH = H // 512  # output column halves for the down projection


def build_moe(Tc: int, mm_dt_name: str = "float32r", use_silu: bool = True):
    """Build the per-core Bass program for Tc tokens (Tc % 512 == 0)."""
    import concourse.bass as bass  # noqa: F401
    import concourse.mybir as mybir
    import concourse.tile as tile
    from concourse import bacc

    assert Tc % TB == 0
    NB = Tc // TB
    f32 = mybir.dt.float32
    mm_dt = getattr(mybir.dt, mm_dt_name)
    Alu = mybir.AluOpType
    Act = mybir.ActivationFunctionType
    X = mybir.AxisListType.X

    nc = bacc.Bacc(
        "TRN2", target_bir_lowering=False, debug=False, num_devices=8
    )

    xT = nc.dram_tensor("xT", [H, Tc], mm_dt, kind="ExternalInput").ap()
    xTr = nc.dram_tensor("xTr", [H, Tc], f32, kind="ExternalInput").ap()
    wg = nc.dram_tensor("wg", [H, I], mm_dt, kind="ExternalInput").ap()
    wu = nc.dram_tensor("wu", [H, I], mm_dt, kind="ExternalInput").ap()
    wd = nc.dram_tensor("wd", [I, H], mm_dt, kind="ExternalInput").ap()
    gpT = nc.dram_tensor("gpT", [H, E], f32, kind="ExternalInput").ap()
    sel = nc.dram_tensor("sel", [P, E], f32, kind="ExternalInput").ap()
    out = nc.dram_tensor("out", [Tc, H], f32, kind="ExternalOutput").ap()

    # Partition-major views: h (or i) split as outer*P + partition
    xT_r = xT.rearrange("(ho p) t -> p ho t", p=P)  # [128, 8, Tc]
    xTr_r = xTr.rearrange("(ho p) t -> p ho t", p=P)  # [128, 8, Tc]
    wg_r = wg.rearrange("(ho p) i -> p ho i", p=P)  # [128, 8, 4096]
    wu_r = wu.rearrange("(ho p) i -> p ho i", p=P)
    wd_r = wd.rearrange("(io p) h -> p io h", p=P)  # [128, 32, 1024]
    gpT_r = gpT.rearrange("(ho p) e -> p ho e", p=P)  # [128, 8, 8]



    with tile.TileContext(nc) as tc:
        with (
            tc.tile_pool(name="singles", bufs=1) as singles,
            tc.tile_pool(name="weights", bufs=1) as wpool,
            tc.tile_pool(name="xp", bufs=2) as xpool,
            tc.tile_pool(name="hp", bufs=2) as hpool,
            tc.tile_pool(name="sp", bufs=4) as spool,
            tc.tile_pool(name="op", bufs=3) as opool,
            tc.tile_pool(name="pgu", bufs=2, space="PSUM") as pgu,
            tc.tile_pool(name="pout", bufs=2, space="PSUM") as pout,
            tc.tile_pool(name="pr", bufs=2, space="PSUM") as pr,
        ):
            gp_sb = singles.tile([P, HO, E], f32)
            nc.scalar.dma_start(gp_sb, gpT_r)
            sel_sb = singles.tile([P, E], f32)
            nc.scalar.dma_start(sel_sb, sel)
            # Per-token renormalized top-2 combine weight for this expert;
            # written once (first weight chunk), read by every chunk.
            w_all = singles.tile([P, NB * TS], f32)

            for sup in range(N_SUPER):
                i0 = sup * IS
                # Per-i-tile weight chunks: compute can start as soon as the
                # first chunk lands, and next-super reloads overlap compute.
                wg_sb, wu_sb, wd_sb = [], [], []
                for it in range(IT):
                    c = i0 + it * P
                    wgc = wpool.tile([P, HO, P], mm_dt, tag=f"wg{it}")
                    nc.sync.dma_start(wgc, wg_r[:, :, c : c + P])
                    wg_sb.append(wgc)
                    wuc = wpool.tile([P, HO, P], mm_dt, tag=f"wu{it}")
                    nc.sync.dma_start(wuc, wu_r[:, :, c : c + P])
                    wu_sb.append(wuc)
                    wdc = wpool.tile([P, H], mm_dt, tag=f"wd{it}")
                    nc.sync.dma_start(wdc, wd_r[:, sup * IT + it, :])
                    wd_sb.append(wdc)

                def down_group(blk, h_sb, grp, sup=sup, wd_sb=wd_sb):
                    # One (token-subtile, output-half) group of the down
                    # projection, back to token-partition layout, scaled by
                    # the combine weight at PSUM eviction; partial sums over
                    # i-chunks accumulate directly in DRAM. Emitted
                    # interleaved with the next block's h production so the
                    # DVE evictions keep PSUM slots recycling.
                    tsub, hh = divmod(grp, HH)
                    col = blk * TS + tsub
                    r0 = blk * TB + tsub * P
                    ops = pout.tile([P, 512], f32, tag="o")
                    for it in range(IT):
                        nc.tensor.matmul(
                            ops,
                            lhsT=h_sb[:, it, tsub * P : (tsub + 1) * P],
                            rhs=wd_sb[it][:, hh * 512 : (hh + 1) * 512],
                            start=(it == 0),
                            stop=(it == IT - 1),
                        )
                    oev = opool.tile([P, 512], f32, tag="oev")
                    nc.vector.tensor_scalar_mul(
                        oev, ops, w_all[:, col : col + 1]
                    )
                    nc.gpsimd.dma_start(
                        out[r0 : r0 + P, hh * 512 : (hh + 1) * 512],
                        oev,
                        accum_op=(Alu.bypass if sup == 0 else Alu.add),
                    )

                pending = None
                for blk in range(NB):
                    t0 = blk * TB
                    x_sb = xpool.tile([P, HO, TB], mm_dt, tag="x")
                    nc.scalar.dma_start(x_sb, xT_r[:, :, t0 : t0 + TB])

                    if sup == 0:
                        # Router: logits -> renormalized top-2 weight for
                        # this core's expert, all in fp32 for exact top-2
                        # selection vs the reference.
                        for tsub in range(TS):
                            col = blk * TS + tsub
                            c0 = t0 + tsub * P
                            xr = xpool.tile([P, HO, P], f32, tag="xr")
                            nc.scalar.dma_start(xr, xTr_r[:, :, c0 : c0 + P])
                            lg = pr.tile([P, E], f32, tag="lg")
                            for ho in range(HO):
                                nc.tensor.matmul(
                                    lg,
                                    lhsT=xr[:, ho, :],
                                    rhs=gp_sb[:, ho, :],
                                    start=(ho == 0),
                                    stop=(ho == HO - 1),
                                )
                            m_sb = spool.tile([P, 1], f32, tag="m")
                            nc.vector.reduce_max(out=m_sb, in_=lg, axis=X)
                            negm = spool.tile([P, 1], f32, tag="negm")
                            nc.vector.reduce_max(
                                out=negm, in_=lg, axis=X, negate=True
                            )
                            q = spool.tile([P, E], f32, tag="q")
                            nc.scalar.activation(q, lg, Act.Exp, bias=negm)
                            # mask of non-top1 entries (l < max)
                            nm = spool.tile([P, E], f32, tag="nm")
                            nc.vector.tensor_scalar(
                                nm, lg, m_sb, None, op0=Alu.is_lt
                            )
                            qm = spool.tile([P, E], f32, tag="qm")
                            nc.vector.tensor_tensor(qm, q, nm, op=Alu.mult)
                            q2 = spool.tile([P, 1], f32, tag="q2")
                            nc.vector.reduce_max(out=q2, in_=qm, axis=X)
                            qs = spool.tile([P, E], f32, tag="qs")
                            nc.vector.tensor_tensor(qs, q, sel_sb, op=Alu.mult)
                            qe = spool.tile([P, 1], f32, tag="qe")
                            nc.vector.reduce_sum(out=qe, in_=qs, axis=X)
                            ind = spool.tile([P, 1], f32, tag="ind")
                            nc.vector.tensor_tensor(ind, qe, q2, op=Alu.is_ge)
                            den = spool.tile([P, 1], f32, tag="den")
                            nc.vector.tensor_scalar_add(den, q2, 1.0)
                            rec = spool.tile([P, 1], f32, tag="rec")
                            nc.vector.reciprocal(rec, den)
                            num = spool.tile([P, 1], f32, tag="num")
                            nc.vector.tensor_tensor(num, qe, ind, op=Alu.mult)
                            nc.vector.tensor_tensor(
                                w_all[:, col : col + 1], num, rec, op=Alu.mult
                            )

                    # Expert FFN for this (i-chunk, token block):
                    # hT[i, t] = silu(Wg.T x)[i, t] * (Wu.T x)[i, t]
                    h_sb = hpool.tile([P, IT, TB], mm_dt, tag="h")
                    for it in range(IT):
                        gps = pgu.tile([P, TB], f32, tag="g")
                        ups = pgu.tile([P, TB], f32, tag="u")
                        for ho in range(HO):
                            nc.tensor.matmul(
                                gps,
                                lhsT=wg_sb[it][:, ho, :],
                                rhs=x_sb[:, ho, :],
                                start=(ho == 0),
                                stop=(ho == HO - 1),
                            )
                        for ho in range(HO):
                            nc.tensor.matmul(
                                ups,
                                lhsT=wu_sb[it][:, ho, :],
                                rhs=x_sb[:, ho, :],
                                start=(ho == 0),
                                stop=(ho == HO - 1),
                            )
                        if use_silu:
                            gs = opool.tile([P, TB], f32, tag="gs")
                            nc.scalar.activation(gs, gps, Act.Silu)
                        else:
                            sig = opool.tile([P, TB], f32, tag="sig")
                            nc.scalar.activation(sig, gps, Act.Sigmoid)
                            gs = opool.tile([P, TB], f32, tag="gs")
                            nc.vector.tensor_tensor(gs, sig, gps, op=Alu.mult)
                        nc.vector.tensor_tensor(
                            h_sb[:, it, :], gs, ups, op=Alu.mult
                        )
                        if pending is not None:
                            down_group(pending[0], pending[1], it)

                    pending = (blk, h_sb)
                if pending is not None:
                    for grp in range(IT):
                        down_group(pending[0], pending[1], grp)

    nc.compile()
    return nc


def _run_spmd(nc, in_maps, trace):
    from concourse import bass_utils

    if trace:
        try:
            res = bass_utils.run_bass_kernel_spmd(
                nc, in_maps, core_ids=list(range(E)), trace=True
            )
            if res.exec_time_ns is not None:
                print(f"HW exec time: {res.exec_time_ns} ns")
            return res
        except Exception as exc:  # fall back to an untraced run
            print(f"traced run failed ({exc!r}); retrying without trace")
    return bass_utils.run_bass_kernel_spmd(
        nc, in_maps, core_ids=list(range(E)), trace=False
    )


def prepare(hidden_states, gate_proj_w, gate_weights, up_weights, down_weights,
            mode="sparse", mm_dt_name="float32r", use_silu=True):
    """Shard inputs per strategy; returns (nc, in_maps, combine_fn)."""
    x = np.ascontiguousarray(hidden_states, dtype=np.float32)
    gpw = np.ascontiguousarray(gate_proj_w, dtype=np.float32)
    T = x.shape[0]
    gpT = np.ascontiguousarray(gpw.T)  # [H, E]
    sel_maps = [
        np.tile(np.eye(E, dtype=np.float32)[e], (P, 1)) for e in range(E)
    ]

    def expert_map(e, xTe):
        return {
            "xT": xTe,
            "xTr": xTe,
            "wg": np.ascontiguousarray(gate_weights[e], np.float32),
            "wu": np.ascontiguousarray(up_weights[e], np.float32),
            "wd": np.ascontiguousarray(down_weights[e], np.float32),
            "gpT": gpT,
            "sel": sel_maps[e],
        }

    if mode == "dense":
        xT = np.ascontiguousarray(x.T)  # [H, T]
        nc = build_moe(T, mm_dt_name, use_silu)
        in_maps = [expert_map(e, xT) for e in range(E)]

        def combine(results):
            outs = [results[e]["out"] for e in range(E)]
            return np.sum(np.stack(outs, axis=0), axis=0).astype(np.float32)

        return nc, in_maps, combine

    # Sparse mode: host-side all-to-all dispatch. Routing decisions here are
    # only used to decide which tokens ship to which expert core; the combine
    # weights themselves are recomputed on device.
    logits = x @ gpw.T  # [T, E] fp32
    top2 = np.argpartition(-logits, 2, axis=1)[:, :2]
    smask = np.zeros((T, E), dtype=bool)
    smask[np.arange(T)[:, None], top2] = True
    idx = [np.nonzero(smask[:, e])[0] for e in range(E)]
    mx = max(len(ix) for ix in idx)
    Tc = max(TB, ((mx + TB - 1) // TB) * TB)

    nc = build_moe(Tc, mm_dt_name, use_silu)
    in_maps = []
    for e in range(E):
        xTe = np.zeros((H, Tc), dtype=np.float32)
        if len(idx[e]):
            xTe[:, : len(idx[e])] = x[idx[e]].T
        in_maps.append(expert_map(e, xTe))

    def combine(results):
        out = np.zeros((T, H), dtype=np.float32)
        for e in range(E):
            n_e = len(idx[e])
            if n_e:
                out[idx[e]] += results[e]["out"][:n_e]
        return out

    return nc, in_maps, combine


def kernel(hidden_states, gate_proj_w, gate_weights, up_weights, down_weights):
    mode = os.environ.get("MOE_MODE", "sparse")
    mm_dt_name = os.environ.get("MOE_MMDT", "float32r")
    trace = os.environ.get("MOE_TRACE", "0") == "1"
    use_silu = os.environ.get("MOE_SILU", "1") == "1"
    nc, in_maps, combine = prepare(
        hidden_states, gate_proj_w, gate_weights, up_weights, down_weights,
        mode=mode, mm_dt_name=mm_dt_name, use_silu=use_silu,
    )
    res = _run_spmd(nc, in_maps, trace)
    return combine(res.results)
